# revision 46
# baseline (speedup 1.0000x reference)
"""Trainium2 Bass kernel for sliding-window unfold (im2col).

reference:  out = x[:, idx, :]  with idx[w, f] = w + f
  x:   [128, 4096, 4]  f32
  out: [128, 4065, 32, 4]  f32

out[b, w] (= 128 floats = 512 B) is the contiguous slice
x[b].flat[4w : 4w + 128]; the problem is a sliding-window byte
replication and HBM write bandwidth is the roofline.  Per core
(16 batches): 33.3 MB of output writes.  On a healthy device this
kernel measures 98.6-100.8 us: ~6.8 us engine preamble, first load
bytes at 8.65 us, all 16 SDMA engines ~fully busy (~81 us each at the
~26.85 GB/s/engine cap; loads+stores = 35.4 MB share the same pipe)
until ~95-97 us, ~3.5 us completion/teardown.  Pairing both compute
engines on batches 1-4 (DVE windows [0,16), ACT [16,31) into one
tile) lifted the 20-30 us window from ~360 to ~435 GB/s vs the
alternating-engine schedule (-1 to -2 us).  The 10-20 us window
(~160-200 GB/s) is load-delivery-bound and is the remaining gap to
the ideal roofline.

Device-state warning (measured 2026-08-10): exec is BIMODAL run to
run.  ~50% of runs are clean (~100.5 us); ~25% have SDMA engine E79
degraded to ~21.5 GB/s (exec 110-118, E79 busy ~97 us vs ~81);
~25% show a ~10% uniform slowdown on all engines (105-113).  The
degradation is NOT caused by this kernel, persists across runs,
accumulates until NRT_EXEC_UNIT_UNRECOVERABLE, and a device reset
clears it (an earlier session's 111 us "baseline" and its E79
doctrine were measured entirely on a degraded device).

SWDGE laws (each verified by dedicated HW probe runs):
  - Descriptors are dealt to the 16 engines in contiguous chunks of
    ceil(N/16), from a global ring cursor g advancing by N + one 4B
    sem-desc per participating engine (SBUF src; D2D adds N only).
    N=120 at g==0 skips one engine; ragged N (N % chunk != 0, e.g.
    113) hits a broken path (all descs on one engine).
  - A store dst must reduce to <= 2 effective dims after merging
    (merged = stride equals inner run, like the tail store's
    [[OB,16],[1536,8],[1,1536]]).  A TRUE 3-dim dst emits ~1k extra
    4B ring descriptors per store and scrambles the spray; a 3-dim
    dst against a non-flat src partially degenerates to 4B descs
    (2x exec).  Proven fast: [[row,N],[1,piece]] 2-dim shapes.
  - 15872 B descriptors run at 591 ns (26.85 GB/s, the engine cap);
    4096/6144/11776 B also full rate; 31744 B run at HALF rate
    (~2255 ns) - do not exceed ~16 KB descriptors.
  - Per-engine dst contiguity/gaps do NOT matter (Y0a's 4 KB descs at
    15.9 KB stride are full rate).
  - An HWDGE queue blocks its engine's instruction FIFO when >~4 DMAs
    are outstanding - keep <= 3 load instructions per ring or the
    engine's later compute ops start 15-20 us late.
  - The Tile scheduler reorders same-queue DMAs by readiness;
    add_dep_helper(sync=False) edges are NOT honored.
  - Tile inserts WAW semaphores between DMAs with overlapping DRAM
    ranges - keep all output writes strictly disjoint.
  - Every dma_start costs ~0.6 us of trigger time on its issuing
    engine; completions fire ~2 us after the last byte.

Measured dead ends (do not retry without new evidence): E79 byte-skew
via 120+8 desc pairs (+1.5-2 us clean, pays only in the ~25% E79-mode
runs, alignment fragile); zero-dep D2D tail stores (strided-src D2D
runs 10-13 GB/s and starves loads, +5 us); SWDGE warmup store,
X0a head-slice load, store reorders, load-ring rebalance (each +1-2
us; the drain start is gated by load traffic, not descriptor
latency); WPP=62 compact-load restructure (load redundancy 2.0->1.5x
= -0.5 MB, but every store shape reachable from 2-batch tiles -
pieces, full rows, 3-dim dst - lands on one of the slow paths above;
best variant 140 us vs 100.5 here); small stores serialize on
completion-sem recycling (~4.5 us/store spacing once >~8 in flight -
36x 1MB stores ran the drain at 250-350 GB/s despite full-rate
packets); loads on the gpsimd SWDGE ring (+6 us: 992 B load descs
are no faster on SWDGE and 1 MB of them ahead of the store descs
delays the early drain; also note a DMA emitted AFTER its readers in
program order gets NO dependency edge - emitting the XG2 load after
the b>=8 expands silently produced garbage, rel err 1.2).
KEPT WINS vs the original checkpoint: paired DVE+ACT expands for
batches 1-4 (20-30 us window 360->435 GB/s) and the V store on the
sync HWDGE queue (Y0a descs ~0.6 us earlier; HWDGE stores are
correct and fine at this size) - together 111.4 -> 99.0-100.7 us
clean, best 98.6.

Layout (per core):
  bulk: partition p holds windows 31p..31p+30 of one batch b.
    load X (248 f32/partition/batch), expand on ACT/DVE into
    Y[128, 3968] via an overlapping-stride read AP, store Y ->
    out[b] windows 0..3967 (contiguous 15.5 KB per partition ->
    128 fat descriptors at 26.5 GB/s/engine).  Batch 0's expand is
    split in half across DVE and ACT so the first store issues ~8.5 us.
  tail: windows 3968..4063 (disjoint from bulk): partition p = 8b+s
    holds 12 consecutive windows of batch b's tail (6 KB descriptors),
    expanded from a tiny raw load.  Window 4064 is contiguous x data:
    a [16, 128] tile (partition = batch) rides a single-engine
    load+store of 512 B descriptors, issued first to prime the pipe.
"""

import numpy as np

from concourse import bacc, mybir, tile
from concourse.bass_utils import run_bass_kernel_spmd

N_CORES = 8
B_FULL = 128
B = B_FULL // N_CORES  # 16 batches per core
S = 4096
C = 4
F = 32
W = S - F + 1    # 4065
FL = F * C       # 128 floats per window
XB = S * C       # 16384 floats per batch of x
OB = W * FL      # 520320 floats per batch of out
WPP = 31         # windows per partition in the bulk store
NBULK = 128 * WPP          # 3968 bulk windows per batch
YROW = WPP * FL            # 3968 floats per partition row
XROW = (WPP - 1) * C + FL  # 248 floats of x per partition per batch

# tail geometry: windows 3968..4063 as 8 slices of 12 windows per batch
# (partition p = 8*b + s, strictly disjoint writes); window 4064 is a
# [16, 128] raw load+store (partition = batch, contiguous 512 B rows).
TSL = 8                    # slices per batch
TWIN = 12                  # windows per slice
TSTR = 12                  # window stride between slices
TROW = TWIN * FL           # 1536 floats of tail output per partition
RLD = 176                  # floats of raw x loaded per partition
W4 = W - 1                 # window 4064
H0 = 8                     # windows in bulk batch-0 first piece (small
                           # so the first store issues ~7.6 us)
H1 = WPP - H0              # 23 windows in second piece (ACT)

_cache = {}


def build_nc():
    nc = bacc.Bacc("TRN2", target_bir_lowering=False)
    x = nc.dram_tensor("x", [B, S, C], mybir.dt.float32, kind="ExternalInput")
    out = nc.dram_tensor("out", [B, W, F, C], mybir.dt.float32, kind="ExternalOutput")

    with tile.TileContext(nc) as tc:
        with (
            tc.tile_pool(name="x01", bufs=2) as x01,
            tc.tile_pool(name="xg1", bufs=1) as xg1p,
            tc.tile_pool(name="xg2", bufs=1) as xg2p,
            tc.tile_pool(name="y0a", bufs=1) as y0ap,
            tc.tile_pool(name="y0b", bufs=1) as y0bp,
            tc.tile_pool(name="yp", bufs=10) as yp,
            tc.tile_pool(name="rp", bufs=1) as rp,
            tc.tile_pool(name="vp", bufs=1) as vp,
            tc.tile_pool(name="tp", bufs=1) as tp,
        ):
            def ld(engine, dst_tile, dst_ap, dst_off, src_ap, src_off):
                src = x[:].copy()
                src.ap = mybir.VecI64Pair(src_ap)
                src.offset = src_off
                dst = dst_tile[:].copy()
                dst.ap = mybir.VecI64Pair(dst_ap)
                dst.offset = dst_off
                engine.dma_start(out=dst, in_=src)

            def st(engine, src_tile, src_ap, src_off, dst_ap, dst_off):
                dst = out[:].copy()
                dst.ap = mybir.VecI64Pair(dst_ap)
                dst.offset = dst_off
                src = src_tile[:].copy()
                src.ap = mybir.VecI64Pair(src_ap)
                src.offset = src_off
                engine.dma_start(out=dst, in_=src)

            def expand(engine, src_tile, src_row, src_off, dst_tile, dst_row,
                       nwin, dst_off=0):
                src = src_tile[:].copy()
                src.ap = mybir.VecI64Pair([[src_row, 128], [C, nwin], [1, FL]])
                src.offset = src_off
                dst = dst_tile[:].copy()
                dst.ap = mybir.VecI64Pair([[dst_row, 128], [FL, nwin], [1, FL]])
                dst.offset = dst_off
                if engine is nc.vector:
                    engine.tensor_copy(out=dst, in_=src)
                else:
                    engine.copy(out=dst, in_=src)

            # ---- loads ----
            # sync ring: batch 0, batch 1, batches 2..7
            X0 = x01.tile([128, XROW], mybir.dt.float32)
            ld(nc.sync, X0, [[XROW, 128], [1, XROW]], 0,
               [[WPP * C, 128], [1, XROW]], 0)
            X1 = x01.tile([128, XROW], mybir.dt.float32)
            ld(nc.sync, X1, [[XROW, 128], [1, XROW]], 0,
               [[WPP * C, 128], [1, XROW]], XB)
            XG1 = xg1p.tile([128, 6 * XROW], mybir.dt.float32)
            ld(nc.sync, XG1, [[6 * XROW, 128], [XROW, 6], [1, XROW]], 0,
               [[WPP * C, 128], [XB, 6], [1, XROW]], 2 * XB)
            # scalar ring: window-4064 raw load (partition = batch), tail
            # raw load, then batches 8..15
            V = vp.tile([16, FL], mybir.dt.float32)
            ld(nc.scalar, V, [[FL, 16], [1, FL]], 0,
               [[XB, B], [1, FL]], W4 * C)
            R = rp.tile([128, RLD], mybir.dt.float32)
            ld(nc.scalar, R, [[RLD, 128], [1, RLD]], 0,
               [[XB, B], [TSTR * C, TSL], [1, RLD]], NBULK * C)
            XG2 = xg2p.tile([128, 8 * XROW], mybir.dt.float32)
            ld(nc.scalar, XG2, [[8 * XROW, 128], [XROW, 8], [1, XROW]], 0,
               [[WPP * C, 128], [XB, 8], [1, XROW]], 8 * XB)

            # ---- expands ----
            # DVE: batch-0 first half, then odd batches (a 2-port-mode
            # DVE copy locks GPSIMD out of the shared SBUF port, so keep
            # DVE's queue clear while the first stores are emitted).
            Y0a = y0ap.tile([128, H0 * FL], mybir.dt.float32)
            expand(nc.vector, X0, XROW, 0, Y0a, H0 * FL, H0)
            # ACT: batch-0 second half, tail expand, then even batches
            Y0b = y0bp.tile([128, H1 * FL], mybir.dt.float32)
            expand(nc.scalar, X0, XROW, H0 * C, Y0b, H1 * FL, H1)

            # Batches 1-4: BOTH engines cooperate per batch (DVE windows
            # [0,16), ACT [16,31) into the same tile; the store waits on
            # both writers).  The clean-run ramp (10-30 us) runs at only
            # ~190-430 GB/s because expanded data isn't ready fast
            # enough; pairing makes Y1..Y4 available ~2 us earlier each.
            # The tail expand moves after the paired halves (its store
            # is reordered past Y4 to avoid head-of-line blocking).
            # Batches 5-15 alternate engines as before.
            HS = 16
            Ys = {}

            def xsrc(b):
                if b == 1:
                    return X1, XROW, 0
                if b < 8:
                    return XG1, 6 * XROW, (b - 2) * XROW
                return XG2, 8 * XROW, (b - 8) * XROW

            for b in range(1, 5):
                src_t, row, off = xsrc(b)
                Y = yp.tile([128, YROW], mybir.dt.float32)
                expand(nc.vector, src_t, row, off, Y, YROW, HS)
                expand(nc.scalar, src_t, row, off + HS * C, Y, YROW,
                       WPP - HS, HS * FL)
                Ys[b] = Y
            T = tp.tile([128, TROW], mybir.dt.float32)
            expand(nc.scalar, R, RLD, 0, T, TROW, TWIN)
            for b in range(5, B):
                src_t, row, off = xsrc(b)
                Y = yp.tile([128, YROW], mybir.dt.float32)
                eng = nc.vector if b % 2 == 1 else nc.scalar
                expand(eng, src_t, row, off, Y, YROW, WPP)
                Ys[b] = Y

            # ---- stores ----
            # window-4064 rides the sync HWDGE queue (idle after its
            # loads; V st waiting its load sem there blocks nothing).
            # Off the gpsimd FIFO head, Y0a's descriptors write ~0.6 us
            # earlier instead of queuing behind V's semaphore wait.
            st(nc.sync, V, [[FL, 16], [1, FL]], 0,
               [[OB, B], [1, FL]], W4 * FL)
            # batch-0 halves, then the tail, then batches 1..15; the
            # queue ends on clean uniform 15.5 KB-descriptor stores.
            st(nc.gpsimd, Y0a, [[H0 * FL, 128], [1, H0 * FL]], 0,
               [[YROW, 128], [1, H0 * FL]], 0)
            st(nc.gpsimd, Y0b, [[H1 * FL, 128], [1, H1 * FL]], 0,
               [[YROW, 128], [1, H1 * FL]], H0 * FL)
            for b in range(1, 5):
                st(nc.gpsimd, Ys[b], [[YROW, 128], [1, YROW]], 0,
                   [[YROW, 128], [1, YROW]], b * OB)
            st(nc.gpsimd, T, [[TROW, 128], [1, TROW]], 0,
               [[OB, B], [TSTR * FL, TSL], [1, TROW]], NBULK * FL)
            for b in range(5, B):
                st(nc.gpsimd, Ys[b], [[YROW, 128], [1, YROW]], 0,
                   [[YROW, 128], [1, YROW]], b * OB)

    nc.finalize()
    return nc


def run_sharded(x: np.ndarray, trace: bool = False):
    """Shard batch across 8 cores, run, gather. Returns (out, raw results)."""
    if "nc" not in _cache:
        _cache["nc"] = build_nc()
    nc = _cache["nc"]

    x = np.ascontiguousarray(x, dtype=np.float32)
    in_maps = [{"x": x[i * B : (i + 1) * B]} for i in range(N_CORES)]
    res = run_bass_kernel_spmd(nc, in_maps, list(range(N_CORES)), trace=trace)
    out = np.concatenate([res.results[i]["out"] for i in range(N_CORES)], axis=0)
    return out, res


def kernel(x: np.ndarray) -> np.ndarray:
    out, _ = run_sharded(x, trace=False)
    return out



# revision 47
# speedup vs baseline: 1.1024x; 1.1024x over previous
"""Trainium2 Bass kernel for sliding-window unfold (im2col).

reference:  out = x[:, idx, :]  with idx[w, f] = w + f
  x:   [128, 4096, 4]  f32
  out: [128, 4065, 32, 4]  f32

out[b, w] (= 128 floats = 512 B) is the contiguous slice
x[b].flat[4w : 4w + 128]; the problem is a sliding-window byte
replication and HBM write bandwidth is the roofline.  Per core
(16 batches): 33.3 MB of output writes.  On a healthy device this
kernel measures 98.6-100.8 us: ~6.8 us engine preamble, first load
bytes at 8.65 us, all 16 SDMA engines ~fully busy (~81 us each at the
~26.85 GB/s/engine cap; loads+stores = 35.4 MB share the same pipe)
until ~95-97 us, ~3.5 us completion/teardown.  Pairing both compute
engines on batches 1-4 (DVE windows [0,16), ACT [16,31) into one
tile) lifted the 20-30 us window from ~360 to ~435 GB/s vs the
alternating-engine schedule (-1 to -2 us).  The 10-20 us window
(~160-200 GB/s) is load-delivery-bound and is the remaining gap to
the ideal roofline.

Device-state warning (measured 2026-08-10): exec is BIMODAL run to
run.  ~50% of runs are clean (~100.5 us); ~25% have SDMA engine E79
degraded to ~21.5 GB/s (exec 110-118, E79 busy ~97 us vs ~81);
~25% show a ~10% uniform slowdown on all engines (105-113).  The
degradation is NOT caused by this kernel, persists across runs,
accumulates until NRT_EXEC_UNIT_UNRECOVERABLE, and a device reset
clears it (an earlier session's 111 us "baseline" and its E79
doctrine were measured entirely on a degraded device).

SWDGE laws (each verified by dedicated HW probe runs):
  - Descriptors are dealt to the 16 engines in contiguous chunks of
    ceil(N/16), from a global ring cursor g advancing by N + one 4B
    sem-desc per participating engine (SBUF src; D2D adds N only).
    N=120 at g==0 skips one engine; ragged N (N % chunk != 0, e.g.
    113) hits a broken path (all descs on one engine).
  - A store dst must reduce to <= 2 effective dims after merging
    (merged = stride equals inner run, like the tail store's
    [[OB,16],[1536,8],[1,1536]]).  A TRUE 3-dim dst emits ~1k extra
    4B ring descriptors per store and scrambles the spray; a 3-dim
    dst against a non-flat src partially degenerates to 4B descs
    (2x exec).  Proven fast: [[row,N],[1,piece]] 2-dim shapes.
  - 15872 B descriptors run at 591 ns (26.85 GB/s, the engine cap);
    4096/6144/11776 B also full rate; 31744 B run at HALF rate
    (~2255 ns) - do not exceed ~16 KB descriptors.
  - Per-engine dst contiguity/gaps do NOT matter (Y0a's 4 KB descs at
    15.9 KB stride are full rate).
  - An HWDGE queue blocks its engine's instruction FIFO when >~4 DMAs
    are outstanding - keep <= 3 load instructions per ring or the
    engine's later compute ops start 15-20 us late.
  - The Tile scheduler reorders same-queue DMAs by readiness;
    add_dep_helper(sync=False) edges are NOT honored.
  - Tile inserts WAW semaphores between DMAs with overlapping DRAM
    ranges - keep all output writes strictly disjoint.
  - Every dma_start costs ~0.6 us of trigger time on its issuing
    engine; completions fire ~2 us after the last byte.

Measured dead ends (do not retry without new evidence): E79 byte-skew
via 120+8 desc pairs (+1.5-2 us clean, pays only in the ~25% E79-mode
runs, alignment fragile); zero-dep D2D tail stores (strided-src D2D
runs 10-13 GB/s and starves loads, +5 us); SWDGE warmup store,
X0a head-slice load, store reorders, load-ring rebalance (each +1-2
us; the drain start is gated by load traffic, not descriptor
latency); WPP=62 compact-load restructure (load redundancy 2.0->1.5x
= -0.5 MB, but every store shape reachable from 2-batch tiles -
pieces, full rows, 3-dim dst - lands on one of the slow paths above;
best variant 140 us vs 100.5 here); small stores serialize on
completion-sem recycling (~4.5 us/store spacing once >~8 in flight -
36x 1MB stores ran the drain at 250-350 GB/s despite full-rate
packets); loads on the gpsimd SWDGE ring (+6 us: 992 B load descs
are no faster on SWDGE and 1 MB of them ahead of the store descs
delays the early drain; also note a DMA emitted AFTER its readers in
program order gets NO dependency edge - emitting the XG2 load after
the b>=8 expands silently produced garbage, rel err 1.2).
KEPT WINS vs the original checkpoint: paired DVE+ACT expands for
batches 1-4 (20-30 us window 360->435 GB/s) and the V store on the
sync HWDGE queue (Y0a descs ~0.6 us earlier; HWDGE stores are
correct and fine at this size) - together 111.4 -> 99.0-100.7 us
clean, best 98.6.

Layout (per core):
  bulk: partition p holds windows 31p..31p+30 of one batch b.
    load X (248 f32/partition/batch), expand on ACT/DVE into
    Y[128, 3968] via an overlapping-stride read AP, store Y ->
    out[b] windows 0..3967 (contiguous 15.5 KB per partition ->
    128 fat descriptors at 26.5 GB/s/engine).  Batch 0's expand is
    split in half across DVE and ACT so the first store issues ~8.5 us.
  tail: windows 3968..4063 (disjoint from bulk): partition p = 8b+s
    holds 12 consecutive windows of batch b's tail (6 KB descriptors),
    expanded from a tiny raw load.  Window 4064 is contiguous x data:
    a [16, 128] tile (partition = batch) rides a single-engine
    load+store of 512 B descriptors, issued first to prime the pipe.
"""

import numpy as np

from concourse import bacc, mybir, tile
from concourse.bass_utils import run_bass_kernel_spmd

N_CORES = 8
B_FULL = 128
B = B_FULL // N_CORES  # 16 batches per core
S = 4096
C = 4
F = 32
W = S - F + 1    # 4065
FL = F * C       # 128 floats per window
XB = S * C       # 16384 floats per batch of x
OB = W * FL      # 520320 floats per batch of out
WPP = 31         # windows per partition in the bulk store
NBULK = 128 * WPP          # 3968 bulk windows per batch
YROW = WPP * FL            # 3968 floats per partition row
XROW = (WPP - 1) * C + FL  # 248 floats of x per partition per batch

# tail geometry: windows 3968..4063 as 8 slices of 12 windows per batch
# (partition p = 8*b + s, strictly disjoint writes); window 4064 is a
# [16, 128] raw load+store (partition = batch, contiguous 512 B rows).
TSL = 8                    # slices per batch
TWIN = 12                  # windows per slice
TSTR = 12                  # window stride between slices
TROW = TWIN * FL           # 1536 floats of tail output per partition
RLD = 176                  # floats of raw x loaded per partition
W4 = W - 1                 # window 4064
H0 = 8                     # windows in bulk batch-0 first piece (small
                           # so the first store issues ~7.6 us)
H1 = WPP - H0              # 23 windows in second piece (ACT)

_cache = {}


def build_nc():
    nc = bacc.Bacc("TRN2", target_bir_lowering=False)
    x = nc.dram_tensor("x", [B, S, C], mybir.dt.float32, kind="ExternalInput")
    out = nc.dram_tensor("out", [B, W, F, C], mybir.dt.float32, kind="ExternalOutput")

    with tile.TileContext(nc) as tc:
        with (
            tc.tile_pool(name="x01", bufs=2) as x01,
            tc.tile_pool(name="xg1", bufs=1) as xg1p,
            tc.tile_pool(name="xg2", bufs=1) as xg2p,
            tc.tile_pool(name="y0a", bufs=1) as y0ap,
            tc.tile_pool(name="y0b", bufs=1) as y0bp,
            tc.tile_pool(name="yp", bufs=10) as yp,
            tc.tile_pool(name="rp", bufs=1) as rp,
            tc.tile_pool(name="vp", bufs=1) as vp,
            tc.tile_pool(name="tp", bufs=1) as tp,
        ):
            def ld(engine, dst_tile, dst_ap, dst_off, src_ap, src_off):
                src = x[:].copy()
                src.ap = mybir.VecI64Pair(src_ap)
                src.offset = src_off
                dst = dst_tile[:].copy()
                dst.ap = mybir.VecI64Pair(dst_ap)
                dst.offset = dst_off
                engine.dma_start(out=dst, in_=src)

            def st(engine, src_tile, src_ap, src_off, dst_ap, dst_off):
                dst = out[:].copy()
                dst.ap = mybir.VecI64Pair(dst_ap)
                dst.offset = dst_off
                src = src_tile[:].copy()
                src.ap = mybir.VecI64Pair(src_ap)
                src.offset = src_off
                engine.dma_start(out=dst, in_=src)

            def expand(engine, src_tile, src_row, src_off, dst_tile, dst_row,
                       nwin, dst_off=0):
                src = src_tile[:].copy()
                src.ap = mybir.VecI64Pair([[src_row, 128], [C, nwin], [1, FL]])
                src.offset = src_off
                dst = dst_tile[:].copy()
                dst.ap = mybir.VecI64Pair([[dst_row, 128], [FL, nwin], [1, FL]])
                dst.offset = dst_off
                if engine is nc.vector:
                    engine.tensor_copy(out=dst, in_=src)
                else:
                    engine.copy(out=dst, in_=src)

            # ---- loads ----
            # sync ring: batch 0, batch 1, batches 2..7
            X0 = x01.tile([128, XROW], mybir.dt.float32)
            ld(nc.sync, X0, [[XROW, 128], [1, XROW]], 0,
               [[WPP * C, 128], [1, XROW]], 0)
            X1 = x01.tile([128, XROW], mybir.dt.float32)
            ld(nc.sync, X1, [[XROW, 128], [1, XROW]], 0,
               [[WPP * C, 128], [1, XROW]], XB)
            XG1 = xg1p.tile([128, 6 * XROW], mybir.dt.float32)
            ld(nc.sync, XG1, [[6 * XROW, 128], [XROW, 6], [1, XROW]], 0,
               [[WPP * C, 128], [XB, 6], [1, XROW]], 2 * XB)
            # scalar ring: window-4064 raw load (partition = batch), tail
            # raw load, then batches 8..15
            V = vp.tile([16, FL], mybir.dt.float32)
            ld(nc.scalar, V, [[FL, 16], [1, FL]], 0,
               [[XB, B], [1, FL]], W4 * C)
            R = rp.tile([128, RLD], mybir.dt.float32)
            ld(nc.scalar, R, [[RLD, 128], [1, RLD]], 0,
               [[XB, B], [TSTR * C, TSL], [1, RLD]], NBULK * C)
            XG2 = xg2p.tile([128, 8 * XROW], mybir.dt.float32)
            ld(nc.scalar, XG2, [[8 * XROW, 128], [XROW, 8], [1, XROW]], 0,
               [[WPP * C, 128], [XB, 8], [1, XROW]], 8 * XB)

            # ---- expands ----
            # DVE: batch-0 first half, then odd batches (a 2-port-mode
            # DVE copy locks GPSIMD out of the shared SBUF port, so keep
            # DVE's queue clear while the first stores are emitted).
            Y0a = y0ap.tile([128, H0 * FL], mybir.dt.float32)
            expand(nc.vector, X0, XROW, 0, Y0a, H0 * FL, H0)
            # ACT: batch-0 second half, tail expand, then even batches
            Y0b = y0bp.tile([128, H1 * FL], mybir.dt.float32)
            expand(nc.scalar, X0, XROW, H0 * C, Y0b, H1 * FL, H1)

            # Batches 1-4: BOTH engines cooperate per batch (DVE windows
            # [0,16), ACT [16,31) into the same tile; the store waits on
            # both writers).  The clean-run ramp (10-30 us) runs at only
            # ~190-430 GB/s because expanded data isn't ready fast
            # enough; pairing makes Y1..Y4 available ~2 us earlier each.
            # The tail expand moves after the paired halves (its store
            # is reordered past Y4 to avoid head-of-line blocking).
            # Batches 5-15 alternate engines as before.
            HS = 16
            Ys = {}

            def xsrc(b):
                if b == 1:
                    return X1, XROW, 0
                if b < 8:
                    return XG1, 6 * XROW, (b - 2) * XROW
                return XG2, 8 * XROW, (b - 8) * XROW

            for b in range(1, 5):
                src_t, row, off = xsrc(b)
                Y = yp.tile([128, YROW], mybir.dt.float32)
                expand(nc.vector, src_t, row, off, Y, YROW, HS)
                expand(nc.scalar, src_t, row, off + HS * C, Y, YROW,
                       WPP - HS, HS * FL)
                Ys[b] = Y
            T = tp.tile([128, TROW], mybir.dt.float32)
            expand(nc.scalar, R, RLD, 0, T, TROW, TWIN)
            for b in range(5, B):
                src_t, row, off = xsrc(b)
                Y = yp.tile([128, YROW], mybir.dt.float32)
                eng = nc.vector if b % 2 == 1 else nc.scalar
                expand(eng, src_t, row, off, Y, YROW, WPP)
                Ys[b] = Y

            # ---- stores ----
            # window-4064 rides the sync HWDGE queue (idle after its
            # loads; V st waiting its load sem there blocks nothing).
            # Off the gpsimd FIFO head, Y0a's descriptors write ~0.6 us
            # earlier instead of queuing behind V's semaphore wait.
            st(nc.sync, V, [[FL, 16], [1, FL]], 0,
               [[OB, B], [1, FL]], W4 * FL)
            # batch-0 halves, then the tail, then batches 1..15; the
            # queue ends on clean uniform 15.5 KB-descriptor stores.
            st(nc.gpsimd, Y0a, [[H0 * FL, 128], [1, H0 * FL]], 0,
               [[YROW, 128], [1, H0 * FL]], 0)
            st(nc.gpsimd, Y0b, [[H1 * FL, 128], [1, H1 * FL]], 0,
               [[YROW, 128], [1, H1 * FL]], H0 * FL)
            # batches 1-4 store in two column pieces matching the two
            # expand halves, so the DVE half's 1 MB ships ~0.8 us before
            # the ACT half completes (8192/7680 B descs, proven sizes).
            for b in range(1, 5):
                st(nc.gpsimd, Ys[b], [[YROW, 128], [1, HS * FL]], 0,
                   [[YROW, 128], [1, HS * FL]], b * OB)
                st(nc.gpsimd, Ys[b], [[YROW, 128], [1, (WPP - HS) * FL]],
                   HS * FL,
                   [[YROW, 128], [1, (WPP - HS) * FL]], b * OB + HS * FL)
            st(nc.gpsimd, T, [[TROW, 128], [1, TROW]], 0,
               [[OB, B], [TSTR * FL, TSL], [1, TROW]], NBULK * FL)
            for b in range(5, B):
                st(nc.gpsimd, Ys[b], [[YROW, 128], [1, YROW]], 0,
                   [[YROW, 128], [1, YROW]], b * OB)

    nc.finalize()
    return nc


def run_sharded(x: np.ndarray, trace: bool = False):
    """Shard batch across 8 cores, run, gather. Returns (out, raw results)."""
    if "nc" not in _cache:
        _cache["nc"] = build_nc()
    nc = _cache["nc"]

    x = np.ascontiguousarray(x, dtype=np.float32)
    in_maps = [{"x": x[i * B : (i + 1) * B]} for i in range(N_CORES)]
    res = run_bass_kernel_spmd(nc, in_maps, list(range(N_CORES)), trace=trace)
    out = np.concatenate([res.results[i]["out"] for i in range(N_CORES)], axis=0)
    return out, res


def kernel(x: np.ndarray) -> np.ndarray:
    out, _ = run_sharded(x, trace=False)
    return out

